# revision 1
# baseline (speedup 1.0000x reference)
"""Trainium2 Bass kernel for nn_MoDBlock (mixture-of-depths transformer block).

Sharding: data-parallel over batch B=8 across the 8 NeuronCores (one batch row
per core; routing/gather/scatter are per-row independent). Everything runs
on-device per core:

  logits  = x @ Wr                      (DVE fused mul+reduce, 32 tiles)
  thr     = 512th-largest logit         (gpsimd kth_largest, exact: desc[511])
  sel, w  = ascending-index compaction  (gpsimd sparse_gather on masked iota /
                                         shifted logits)
  tok     = dma_gather(x, sel)          (512 rows of 4KB)
  block   = pre-LN attention + SwiGLU MLP, bf16 matmuls with f32 accumulation,
            f32 LN/softmax statistics; softmax without max-subtraction
            (|scores/8| < 3 for this operator's scale), causal mask applied
            multiplicatively after exp.
  out     = copy of x, then dma_scatter_add(out, proc * w, sel)

Host-side preprocessing: weights pre-cast to bf16 (halves weight HBM traffic),
LN gains folded into Wqkv/W1/W3 rows.
"""

import os
import numpy as np
import ml_dtypes

import concourse.bass as bass
import concourse.mybir as mybir
import concourse.tile as tile
from concourse import bacc, masks
from concourse.bass_utils import run_bass_kernel_spmd

F32 = mybir.dt.float32
BF16 = mybir.dt.bfloat16
I16 = mybir.dt.int16
I32 = mybir.dt.int32
U32 = mybir.dt.uint32
AF = mybir.ActivationFunctionType
ALU = mybir.AluOpType

B, T, C = 8, 4096, 1024
H, DH, FF = 16, 64, 4096
K = 512                      # routed tokens per batch row
EPS = 1e-5
NT = T // 128                # 32 x-tiles
NI = K // 128                # 4 reduced-seq token chunks
NCC = C // 128               # 8 feature chunks
NFC = FF // 128              # 32 ffn chunks
N_CORES = 8


KSTOP = int(os.environ.get("KSTOP", "99"))


def build(nc, tc, es):
    x_d = nc.dram_tensor("x", (T, C), F32, kind="ExternalInput").ap()
    wr_d = nc.dram_tensor("wr", (1, C), F32, kind="ExternalInput").ap()
    wqkv_d = nc.dram_tensor("wqkv", (C, 3 * C), BF16, kind="ExternalInput").ap()
    wo_d = nc.dram_tensor("wo", (C, C), BF16, kind="ExternalInput").ap()
    w1_d = nc.dram_tensor("w1", (C, FF), BF16, kind="ExternalInput").ap()
    w3_d = nc.dram_tensor("w3", (C, FF), BF16, kind="ExternalInput").ap()
    w2_d = nc.dram_tensor("w2", (FF, C), BF16, kind="ExternalInput").ap()
    out_d = nc.dram_tensor("out", (T, C), F32, kind="ExternalOutput").ap()
    wb_d = nc.dram_tensor("w_bounce", (K,), F32).ap()
    ib_d = nc.dram_tensor("idx_bounce", (K,), I16).ap()

    const = es.enter_context(tc.tile_pool(name="const", bufs=1))
    ident = const.tile([128, 128], BF16)
    masks.make_identity(nc, ident[:])
    ones65 = const.tile([65, 128], BF16)
    nc.vector.memset(ones65[:], 1.0)
    # causal masks per j-chunk: cmask[jc][p, i] = 1.0 if i >= jc*128+p else 0
    cmask = []
    for jc in range(NI):
        cm = const.tile([128, K], BF16, name=f"cmask{jc}")
        nc.gpsimd.memset(cm[:], 1.0)
        nc.gpsimd.affine_select(
            out=cm[:], in_=cm[:], compare_op=ALU.is_ge, fill=0.0,
            base=-jc * 128, channel_multiplier=-1, pattern=[[1, K]],
        )
        cmask.append(cm)
    # register const APs used as activation biases (Exp/Silu need 0.0, Sqrt
    # uses EPS); bass converts float biases via nc.const_aps
    for val in (0.0, EPS):
        cz = const.tile([128, 1], F32, name=f"constap_{val}")
        nc.vector.memset(cz[:], val)
        nc.const_aps.aps[(F32, val)] = cz[:]
    wr_b = const.tile([128, C], F32)
    nc.sync.dma_start(out=wr_b[0:1, :], in_=wr_d[:, :])
    nc.gpsimd.partition_broadcast(wr_b[:], wr_b[0:1, :])
    logit_sb = const.tile([128, NT], F32)       # token t = col*128 + p

    # persistent activations
    py = es.enter_context(tc.tile_pool(name="py", bufs=1))
    y0 = py.tile([128, NI, C], F32)             # gathered rows, token-major
    y1 = py.tile([128, NI, C], F32)             # after attention residual
    swr = py.tile([128, NI, C], F32)            # (y2)*w, scatter source
    idx128 = py.tile([128, 32], I16)
    w128 = py.tile([128, NI], F32)

    # ---------------- stage 1: x load, logits, x copy-through --------------
    with tc.tile_pool(name="xio", bufs=6) as xio, \
         tc.tile_pool(name="junkp", bufs=2) as junkp:
        for t in range(NT):
            xt = xio.tile([128, C], F32, tag="xt")
            nc.sync.dma_start(out=xt[:], in_=x_d[t * 128:(t + 1) * 128, :])
            junk = junkp.tile([128, C], BF16, tag="junk")
            nc.vector.scalar_tensor_tensor(
                out=junk[:], in0=xt[:], scalar=1.0, in1=wr_b[:],
                op0=ALU.mult, op1=ALU.mult,
                accum_out=logit_sb[:, t:t + 1])

    if KSTOP == 1:
        nc.sync.dma_start(out=out_d[0:128, 0:NT], in_=logit_sb[:])
        return

    # ---------------- stage 2: routing ----------------
    rt = es.enter_context(tc.tile_pool(name="route", bufs=1))
    kth = rt.tile([1, 2], F32)
    # quantile s.t. k_adj = floor(0.1246*4095) = 510 -> out[0,1] = desc[511]
    nc.gpsimd.kth_largest(kth[:], logit_sb[:], n_per_lane=NT, k=510,
                          quantile=1.0 - 0.1246)
    thr16 = rt.tile([16, 1], F32)
    nc.gpsimd.partition_broadcast(thr16[:], kth[0:1, 1:2])

    # rearrange logits to 16-wrapped: l16[p, 8f+g] = logit_sb[16g+p, f]
    l16 = rt.tile([16, 256], F32)
    for g in range(8):
        nc.sync.dma_start(out=l16[:, g::8],
                          in_=logit_sb[g * 16:(g + 1) * 16, :])
    m01 = rt.tile([16, 256], F32)
    nc.vector.tensor_scalar(out=m01[:], in0=l16[:], scalar1=thr16[:, 0:1],
                            scalar2=None, op0=ALU.is_ge)
    iota_i = rt.tile([16, 256], I32)
    nc.gpsimd.iota(iota_i[:], pattern=[[16, 256]], base=1, channel_multiplier=1)
    iota_f = rt.tile([16, 256], F32)
    nc.vector.tensor_copy(iota_f[:], iota_i[:])
    selm = rt.tile([16, 256], F32)   # j+1 if selected else 0 ... then -1
    nc.vector.tensor_tensor(out=selm[:], in0=m01[:], in1=iota_f[:], op=ALU.mult)
    nc.vector.tensor_scalar_add(selm[:], selm[:], -1.0)
    wcand = rt.tile([16, 256], F32)  # logit+99 if selected else -1
    nc.vector.scalar_tensor_tensor(out=wcand[:], in0=l16[:], scalar=100.0,
                                   in1=m01[:], op0=ALU.add, op1=ALU.mult)
    nc.vector.tensor_scalar_add(wcand[:], wcand[:], -1.0)

    idxw = rt.tile([16, 32], F32)
    wsel = rt.tile([16, 32], F32)
    nfound = rt.tile([1, 1], U32)
    nfound2 = rt.tile([1, 1], U32)
    nc.gpsimd.sparse_gather(idxw[:], selm[:], num_found=nfound[:])
    nc.gpsimd.sparse_gather(wsel[:], wcand[:], num_found=nfound2[:])
    nc.vector.tensor_scalar_add(wsel[:], wsel[:], -99.0)
    idxw16 = rt.tile([16, 32], I16)
    nc.vector.tensor_copy(idxw16[:], idxw[:])

    # bounce to DRAM in linear j order; reload in the layouts we need
    nc.sync.dma_start(out=ib_d.rearrange("(f p) -> p f", p=16), in_=idxw16[:])
    nc.sync.dma_start(out=wb_d.rearrange("(f p) -> p f", p=16), in_=wsel[:])
    for g in range(8):
        nc.sync.dma_start(out=idx128[g * 16:(g + 1) * 16, :],
                          in_=ib_d.rearrange("(f p) -> p f", p=16))
    nc.sync.dma_start(out=w128[:], in_=wb_d.rearrange("(i p) -> p i", p=128))

    # ---------------- stage 3: gather + LN1 + transpose ----------------
    nc.gpsimd.dma_gather(out_ap=y0[:], in_ap=x_d[:, :], idxs_ap=idx128[:],
                         num_idxs=K, num_idxs_reg=K, elem_size=C)

    if KSTOP == 3:
        for c in range(NI):
            nc.sync.dma_start(out=out_d[c * 128:(c + 1) * 128, :],
                              in_=y0[:, c, :])
        nc.sync.dma_start(out=out_d[512:640, 0:NI], in_=w128[:])
        return

    lnp = es.enter_context(tc.tile_pool(name="lnp", bufs=4))

    def ln_tokmajor(src, dst):
        # LayerNorm over free dim (C) of token-major [128, C] f32 -> bf16
        st = lnp.tile([128, 2, 6], F32, tag="bnst")
        nc.vector.bn_stats(st[:, 0, :], src[:, 0:512])
        nc.vector.bn_stats(st[:, 1, :], src[:, 512:1024])
        ag = lnp.tile([128, 2], F32, tag="bnag")
        nc.vector.bn_aggr(ag[:], st[:])
        sd = lnp.tile([128, 1], F32, tag="sd")
        nc.scalar.activation(sd[:], ag[:, 1:2], AF.Sqrt, bias=EPS)
        rs = lnp.tile([128, 1], F32, tag="rs")
        nc.vector.reciprocal(rs[:], sd[:])
        nb = lnp.tile([128, 1], F32, tag="nb")
        nc.vector.scalar_tensor_tensor(out=nb[:], in0=ag[:, 0:1], scalar=-1.0,
                                       in1=rs[:], op0=ALU.mult, op1=ALU.mult)
        nc.scalar.activation(dst[:], src[:], AF.Identity, bias=nb[:],
                             scale=rs[:])

    from contextlib import ExitStack as _ES
    pq_stack = _ES()
    pqkv = pq_stack.enter_context(tc.tile_pool(name="pqkv", bufs=1))
    hT = [pqkv.tile([128, K], BF16, name=f"hT{cc}") for cc in range(NCC)]
    qkv_sb = [pqkv.tile([128, K], BF16, name=f"qkv{m}") for m in range(24)]
    o16 = [pqkv.tile([64, K], BF16, name=f"o{h}") for h in range(H)]

    with tc.tile_pool(name="hbuf", bufs=2) as hbuf, \
         tc.tile_pool(name="tpsum", bufs=4, space="PSUM") as tpsum:
        for i in range(NI):
            hti = hbuf.tile([128, C], BF16, tag="h")
            ln_tokmajor(y0[:, i, :], hti[:])
            for cc in range(NCC):
                pt = tpsum.tile([128, 128], BF16, tag="tp")
                nc.tensor.transpose(pt[:], hti[:, cc * 128:(cc + 1) * 128],
                                    ident[:])
                nc.vector.tensor_copy(hT[cc][:, i * 128:(i + 1) * 128], pt[:])

    # ---------------- stage 4: QKV projection ----------------
    with tc.tile_pool(name="wqkvp", bufs=1) as wqkvp, \
         tc.tile_pool(name="qpsum", bufs=3, space="PSUM") as qpsum:
        wq = []
        for cc in range(NCC):
            wt = wqkvp.tile([128, 3 * C], BF16, tag=f"wq{cc}")
            nc.sync.dma_start(out=wt[:], in_=wqkv_d[cc * 128:(cc + 1) * 128, :])
            wq.append(wt)
        for m in range(24):
            pq = qpsum.tile([128, K], F32, tag="pq")
            for cc in range(NCC):
                nc.tensor.matmul(pq[:], wq[cc][:, m * 128:(m + 1) * 128],
                                 hT[cc][:], start=(cc == 0), stop=(cc == 7))
            nc.any.tensor_copy(qkv_sb[m][:], pq[:])

    if KSTOP == 4:
        for m in range(24):
            nc.gpsimd.dma_start(out=out_d[m * 128:(m + 1) * 128, 0:K],
                                in_=qkv_sb[m][:])
        return

    # deferred x copy-through: interleaved into the attention loop below so
    # its DMA bandwidth uses block-compute idle time instead of competing
    # with the routing-critical x load
    xcp = pq_stack.enter_context(tc.tile_pool(name="xcp", bufs=4))

    def emit_xcopy(t):
        xt2 = xcp.tile([128, C], F32, tag="xc")
        nc.sync.dma_start(out=xt2[:], in_=x_d[t * 128:(t + 1) * 128, :])
        nc.sync.dma_start(out=out_d[t * 128:(t + 1) * 128, :], in_=xt2[:])

    # ---------------- stage 5: attention ----------------
    # layouts: q = qkv chunks 0-7, k = 8-15, v = 16-23; head h lives in chunk
    # h//2 at partition offset 64*(h%2).
    with tc.tile_pool(name="apool", bufs=6) as ap_, \
         tc.tile_pool(name="spsum", bufs=3, space="PSUM") as spsum, \
         tc.tile_pool(name="vpsum", bufs=2, space="PSUM") as vpsum, \
         tc.tile_pool(name="opsum", bufs=2, space="PSUM") as opsum, \
         tc.tile_pool(name="zpsum", bufs=1, space="PSUM") as zpsum:
        for h in range(H):
            qch, po = h // 2, 64 * (h % 2)
            q_sl = qkv_sb[qch][po:po + 64, :]
            k_sl = qkv_sb[8 + qch][po:po + 64, :]
            v_sl = qkv_sb[16 + qch][po:po + 64, :]
            att = []
            for jc in range(NI):
                # causal skip: queries i < jc*128 are fully masked for this
                # j-chunk; compute only the live i-range
                lo = jc * 128
                ps = spsum.tile([128, K], F32, tag="ps")
                nc.tensor.matmul(ps[:, lo:], k_sl[:, jc * 128:(jc + 1) * 128],
                                 q_sl[:, lo:], start=True, stop=True)
                ea = ap_.tile([128, K], BF16, tag="ea")
                nc.scalar.activation(ea[:, lo:], ps[:, lo:], AF.Exp,
                                     scale=0.125)
                am = ap_.tile([128, K], BF16, tag="am")
                nc.vector.tensor_tensor(out=am[:, lo:], in0=ea[:, lo:],
                                        in1=cmask[jc][:, lo:], op=ALU.mult)
                att.append(am)
            po_t = opsum.tile([65, K], F32, tag="po")
            for jc in range(NI):
                lo = jc * 128
                pv = vpsum.tile([128, 64], BF16, tag="pv")
                nc.tensor.transpose(pv[:], v_sl[:, jc * 128:(jc + 1) * 128],
                                    ident[po:po + 64, po:po + 64])
                vte = ap_.tile([128, 72], BF16, tag="vte")
                nc.vector.tensor_copy(vte[:, 0:64], pv[:])
                nc.vector.memset(vte[:, 64:65], 1.0)
                nc.tensor.matmul(po_t[:, lo:], vte[:, 0:65], att[jc][:, lo:],
                                 start=(jc == 0), stop=(jc == 3))
            # evict raw o (divide by Z after broadcast)
            orw = ap_.tile([64, K], BF16, tag="orw")
            nc.scalar.activation(orw[:], po_t[0:64, :], AF.Copy)
            # Z strip lives at partition 64 of po_t; stay lane-aligned
            pz = zpsum.tile([64, K], F32, tag="pz")
            zr = ap_.tile([65, K], F32, tag="zr")
            nc.vector.reciprocal(zr[64:65, :], po_t[64:65, :])
            zrb = ap_.tile([65, K], BF16, tag="zrb")
            nc.vector.tensor_copy(zrb[64:65, :], zr[64:65, :])
            nc.tensor.matmul(pz[:], ones65[64:65, 0:64], zrb[64:65, :],
                             start=True, stop=True)
            nc.vector.tensor_tensor(out=o16[h][:], in0=orw[:], in1=pz[:],
                                    op=ALU.mult)
            emit_xcopy(2 * h)
            emit_xcopy(2 * h + 1)

    if KSTOP == 5:
        for h in range(H):
            nc.gpsimd.dma_start(out=out_d[h * 64:(h + 1) * 64, 0:K],
                                in_=o16[h][:])
        return

    # ---------------- stage 6: Wo (moving) + residual ----------------
    with tc.tile_pool(name="wop", bufs=1) as wop, \
         tc.tile_pool(name="aopsum", bufs=2, space="PSUM") as aopsum:
        wo_sb = []
        for cd in range(16):
            wt = wop.tile([64, C], BF16, tag=f"wo{cd}")
            nc.sync.dma_start(out=wt[:], in_=wo_d[cd * 64:(cd + 1) * 64, :])
            wo_sb.append(wt)
        for i in range(NI):
            pao = aopsum.tile([128, C], F32, tag="pao")
            for cd in range(16):          # o-feature chunks of 64 (heads)
                rhs = wo_sb[cd]
                for nh in range(2):
                    nc.tensor.matmul(
                        pao[:, nh * 512:(nh + 1) * 512],
                        o16[cd][:, i * 128:(i + 1) * 128],
                        rhs[:, nh * 512:(nh + 1) * 512],
                        start=(cd == 0), stop=(cd == 15))
            nc.vector.tensor_tensor(out=y1[:, i, :], in0=pao[:],
                                    in1=y0[:, i, :], op=ALU.add)

    if KSTOP == 6:
        for c in range(NI):
            nc.sync.dma_start(out=out_d[c * 128:(c + 1) * 128, :],
                              in_=y1[:, c, :])
        return
    pq_stack.close()

    # ---------------- stage 7: LN2 + transpose ----------------
    pmlp = es.enter_context(tc.tile_pool(name="pmlp", bufs=1))
    mT = [pmlp.tile([128, K], BF16, name=f"mT{cc}") for cc in range(NCC)]
    h2 = [pmlp.tile([128, K], BF16, name=f"h2{f}") for f in range(NFC)]
    with tc.tile_pool(name="mbuf", bufs=2) as mbuf, \
         tc.tile_pool(name="tpsum2", bufs=4, space="PSUM") as tpsum2:
        for i in range(NI):
            mti = mbuf.tile([128, C], BF16, tag="m")
            ln_tokmajor(y1[:, i, :], mti[:])
            for cc in range(NCC):
                pt = tpsum2.tile([128, 128], BF16, tag="tp2")
                nc.tensor.transpose(pt[:], mti[:, cc * 128:(cc + 1) * 128],
                                    ident[:])
                nc.vector.tensor_copy(mT[cc][:, i * 128:(i + 1) * 128], pt[:])

    # ---------------- stage 8: W1/W3 + SwiGLU ----------------
    with tc.tile_pool(name="w13p", bufs=2) as w13p, \
         tc.tile_pool(name="upsum", bufs=2, space="PSUM") as upsum, \
         tc.tile_pool(name="gpsum", bufs=2, space="PSUM") as gpsum, \
         tc.tile_pool(name="sbuf8", bufs=3) as sbuf8:
        for fg in range(4):               # groups of 8 ffn chunks
            w1g, w3g = [], []
            for cc in range(NCC):
                t1 = w13p.tile([128, 1024], BF16, tag=f"w1g{cc}")
                nc.sync.dma_start(
                    out=t1[:],
                    in_=w1_d[cc * 128:(cc + 1) * 128,
                             fg * 1024:(fg + 1) * 1024])
                w1g.append(t1)
                t3 = w13p.tile([128, 1024], BF16, tag=f"w3g{cc}")
                nc.sync.dma_start(
                    out=t3[:],
                    in_=w3_d[cc * 128:(cc + 1) * 128,
                             fg * 1024:(fg + 1) * 1024])
                w3g.append(t3)
            for fi in range(8):
                f = fg * 8 + fi
                pu = upsum.tile([128, K], F32, tag="pu")
                pg = gpsum.tile([128, K], F32, tag="pg")
                for cc in range(NCC):
                    nc.tensor.matmul(pu[:],
                                     w1g[cc][:, fi * 128:(fi + 1) * 128],
                                     mT[cc][:], start=(cc == 0),
                                     stop=(cc == 7))
                for cc in range(NCC):
                    nc.tensor.matmul(pg[:],
                                     w3g[cc][:, fi * 128:(fi + 1) * 128],
                                     mT[cc][:], start=(cc == 0),
                                     stop=(cc == 7))
                sg = sbuf8.tile([128, K], BF16, tag="sg")
                nc.scalar.activation(sg[:], pu[:], AF.Sigmoid)
                us = sbuf8.tile([128, K], BF16, tag="us")
                nc.vector.scalar_tensor_tensor(out=us[:], in0=pu[:],
                                               scalar=1.0, in1=sg[:],
                                               op0=ALU.mult, op1=ALU.mult)
                nc.vector.tensor_tensor(out=h2[f][:], in0=us[:], in1=pg[:],
                                        op=ALU.mult)

    if KSTOP == 8:
        for f in range(8):
            nc.gpsimd.dma_start(out=out_d[f * 128:(f + 1) * 128, 0:K],
                                in_=h2[f][:])
        return

    # ---------------- stage 9: W2 (moving) + residual + w-scale ------------
    with tc.tile_pool(name="w2p", bufs=1) as w2p, \
         tc.tile_pool(name="mpsum", bufs=2, space="PSUM") as mpsum, \
         tc.tile_pool(name="y2buf", bufs=2) as y2buf:
        w2_sb = []
        for f in range(NFC):
            wt = w2p.tile([128, C], BF16, tag=f"w2_{f}")
            nc.sync.dma_start(out=wt[:], in_=w2_d[f * 128:(f + 1) * 128, :])
            w2_sb.append(wt)
        for i in range(NI):
            pm = mpsum.tile([128, C], F32, tag="pm")
            for f in range(NFC):
                for nh in range(2):
                    nc.tensor.matmul(
                        pm[:, nh * 512:(nh + 1) * 512],
                        h2[f][:, i * 128:(i + 1) * 128],
                        w2_sb[f][:, nh * 512:(nh + 1) * 512],
                        start=(f == 0), stop=(f == 31))
            y2t = y2buf.tile([128, C], F32, tag="y2")
            nc.vector.tensor_tensor(out=y2t[:], in0=pm[:], in1=y1[:, i, :],
                                    op=ALU.add)
            nc.scalar.activation(swr[:, i, :], y2t[:], AF.Copy,
                                 scale=w128[:, i:i + 1])

    # ---------------- stage 10: scatter-add ----------------
    nc.gpsimd.dma_scatter_add(out_ap=out_d[:, :], in_ap=swr[:],
                              idxs_ap=idx128[:], num_idxs=K, num_idxs_reg=K,
                              elem_size=C)


_CACHE = {}


def _get_compiled():
    if "nc" in _CACHE:
        return _CACHE["nc"]
    from contextlib import ExitStack
    nc = bacc.Bacc("TRN2", target_bir_lowering=False, debug=False)
    with tile.TileContext(nc) as tc:
        with ExitStack() as es:
            build(nc, tc, es)
    nc.compile()
    _CACHE["nc"] = nc
    return nc


def kernel(**inputs):
    nc = _get_compiled()
    x = np.asarray(inputs["x"], dtype=np.float32)          # (8, 4096, 1024)
    Wr = np.asarray(inputs["Wr"], dtype=np.float32)
    ln1_g = np.asarray(inputs["ln1_g"], dtype=np.float32)
    ln2_g = np.asarray(inputs["ln2_g"], dtype=np.float32)
    bf = ml_dtypes.bfloat16
    wqkv = (np.asarray(inputs["Wqkv"], np.float32)
            * ln1_g[:, None]).astype(bf)
    wo = np.asarray(inputs["Wo"], np.float32).astype(bf)
    w1 = (np.asarray(inputs["W1"], np.float32) * ln2_g[:, None]).astype(bf)
    w3 = (np.asarray(inputs["W3"], np.float32) * ln2_g[:, None]).astype(bf)
    w2 = np.asarray(inputs["W2"], np.float32).astype(bf)
    shared = {
        "wr": np.ascontiguousarray(Wr[None, :]),
        "wqkv": np.ascontiguousarray(wqkv),
        "wo": np.ascontiguousarray(wo),
        "w1": np.ascontiguousarray(w1),
        "w3": np.ascontiguousarray(w3),
        "w2": np.ascontiguousarray(w2),
    }
    in_maps = [{"x": np.ascontiguousarray(x[b]), **shared} for b in range(B)]
    res = run_bass_kernel_spmd(nc, in_maps, core_ids=list(range(N_CORES)))
    _CACHE["last_results"] = res
    out = np.stack([res.results[b]["out"] for b in range(B)], axis=0)
    return out.astype(np.float32)

